# revision 2
# baseline (speedup 1.0000x reference)
"""Bi-Real Net binary conv2d (3x3, pad 1, stride 1) for Trainium2, 8 NeuronCores.

Math (forward values of the reference):
    xb = sign(x)                      in {-1, 0, +1}
    scale[o] = mean_{i,kh,kw} |w[o,i,kh,kw]|
    wb = scale[o] * sign(w)
    y = conv2d_NCHW(xb, wb, pad=1)

v2 kernel strategy (one core = 4 images, data-parallel over batch):
    - Per image: DMA [128, 28, 112] f32 pieces -> SBUF, ACT Sign ->
      zero-padded fp8 buffer [128, 115, 128] (pitch 128; cols >=114 and
      garbage PSUM columns are never read).
    - Conv as 6 DoubleRow fp8 matmuls per 4-output-row chunk (N=448,
      4-dim rhs APs):
        * (kh=0,kw)+(kh=1,kw) pairs, pair step = pitch   (3 matmuls)
        * (kh=2,kw)+ZERO pairs (odd weight rows zeroed)  (3 matmuls)
      All matmuls run at the 0.5 cycles/row DoubleRow rate.
    - LDWEIGHTS amortized: chunks processed in groups of 4 (8 PSUM banks,
      alternating half-sets); per group each of the 6 weight tiles is
      loaded once (standalone ldweights) and applied to all 4 chunks via
      non-self-loading matmuls (inst.ldweights=False).
    - PSUM evacuation on DVE multiplies by per-channel scale[o] and
      writes fp16 stage tiles; output DMA'd as fp16 (halves output HBM
      traffic; |y| = scale*int with rel quant err 2^-11 << the 2e-2
      tolerance) and upcast to f32 on the host.
"""

import sys

sys.path.insert(0, "/opt/trn_rl_repo")

import numpy as np

import concourse.bacc as bacc
import concourse.bass as bass
import concourse.mybir as mybir
import concourse.tile as tile
from concourse.bass_utils import run_bass_kernel_spmd
from concourse.masks import make_identity

N_CORES = 8
B, C, H, W = 32, 128, 112, 112
BL = B // N_CORES  # images per core
HP = H + 2  # padded height (114)

F32 = mybir.dt.float32
F16 = mybir.dt.float16
BF16 = mybir.dt.bfloat16
FP8 = mybir.dt.float8e4
DR = mybir.MatmulPerfMode.DoubleRow

RP = 128  # fp8 padded-row pitch (DoubleRow pair step, must be %16)
NROWS = HP + 1  # 115: bottom zero row absorbs the kh=2 odd-pair overrun
N_LOADROWS = 28
N_SIGNROWS = 14
N_STAGEROWS = 28  # 7 chunks per stage
GROUP = 4  # chunks per ldweights group (2 groups in flight on 8 banks)

VARIANT = "v2"


def build_nc_v2(variant="v2"):
    nc = bacc.Bacc(
        "TRN2", target_bir_lowering=False, debug=False, num_devices=N_CORES
    )
    x = nc.declare_dram_parameter("x", [BL, C, H, W], F32, isOutput=False)
    w = nc.declare_dram_parameter("weight", [C, C, 3, 3], F32, isOutput=False)
    y = nc.declare_dram_parameter("y", [BL, C, H, W], F16, isOutput=True)

    with tile.TileContext(nc) as tc:
        with (
            tc.tile_pool(name="consts", bufs=1) as consts,
            tc.tile_pool(name="raw", bufs=1) as raw_pool,
            tc.tile_pool(name="xpad", bufs=1) as xpad_pool,
            tc.tile_pool(name="stage", bufs=1) as stage_pool,
        ):
            # wdr[i, kw, j, o]: j=0 -> tap (0,kw), j=1 -> tap (1,kw)
            wdr = consts.tile([C, 3, 2, C], FP8)
            # w2z[i, kw, j, o]: j=0 -> tap (2,kw), j=1 -> zeros
            w2z = consts.tile([C, 3, 2, C], FP8)
            scale = consts.tile([C, 1], F32)
            identity = consts.tile([C, C], BF16)

            # ---- weight prep (transient pools; PSUM freed before main) ----
            with (
                tc.tile_pool(name="wprep", bufs=1) as wp,
                tc.tile_pool(name="wpsum", bufs=1, space="PSUM") as wpsum,
            ):
                wf = wp.tile([C, C, 3, 3], F32)
                nc.sync.dma_start(wf[:, :, :, :], w[:, :, :, :])
                # prefetch image 0 while weights are prepped
                raws0 = []
                for li in range(H // N_LOADROWS):
                    raw = raw_pool.tile(
                        [C, N_LOADROWS, W], F32, tag="raw", bufs=4, name="raw"
                    )
                    nc.sync.dma_start(
                        raw[:, :, :],
                        x[0][:, li * N_LOADROWS : (li + 1) * N_LOADROWS, :],
                    )
                    raws0.append(raw)
                make_identity(nc, identity)
                nc.vector.memset(w2z[:, :, 1, :], 0.0)
                wabs = wp.tile([C, C, 3, 3], F32)
                ssum = wp.tile([C, 1], F32)
                nc.scalar.activation(
                    wabs[:, :, :, :],
                    wf[:, :, :, :],
                    mybir.ActivationFunctionType.Abs,
                    accum_out=ssum[:, :],
                )
                nc.scalar.mul(scale[:, :], ssum[:, :], 1.0 / (C * 9))
                wsign = wp.tile([C, C, 3, 3], BF16)
                nc.scalar.sign(wsign[:, :, :, :], wf[:, :, :, :])
                for kh in range(3):
                    for kw in range(3):
                        pst = wpsum.tile([C, C], BF16, tag="pst", bufs=2, name="pst")
                        nc.tensor.transpose(
                            pst[:, :], wsign[:, :, kh, kw], identity[:, :]
                        )
                        if kh < 2:
                            dst = wdr[:, kw, kh, :]
                        else:
                            dst = w2z[:, kw, 0, :]
                        nc.vector.tensor_copy(dst, pst[:, :])

            # ---- padded activation buffers (double-buffered across images) --
            def border_memsets(xp):
                nc.gpsimd.memset(xp[:, 0, 0:114], 0.0)
                nc.gpsimd.memset(xp[:, 113:115, 0:114], 0.0)
                nc.gpsimd.memset(xp[:, 1:113, 0], 0.0)
                nc.gpsimd.memset(xp[:, 1:113, 113], 0.0)

            xpads = []
            for k in range(2):
                xp = xpad_pool.tile(
                    [C, NROWS, RP], FP8, tag=f"xpad{k}", name=f"xpad{k}"
                )
                xpads.append(xp)
            border_memsets(xpads[0])  # buf 1 deferred past image 0's signs

            with tc.tile_pool(name="psum", bufs=1, space="PSUM") as psum_pool:
                n_chunks = H // 4
                for n in range(BL):
                    xim = x[n]
                    yim = y[n]
                    xpad = xpads[n % 2]
                    # loads + signs
                    for li in range(H // N_LOADROWS):
                        r0 = li * N_LOADROWS
                        if n == 0:
                            raw = raws0[li]
                        else:
                            raw = raw_pool.tile(
                                [C, N_LOADROWS, W], F32, tag="raw", bufs=4,
                                name="raw",
                            )
                            nc.sync.dma_start(
                                raw[:, :, :], xim[:, r0 : r0 + N_LOADROWS, :]
                            )
                        for a in range(0, N_LOADROWS, N_SIGNROWS):
                            rr = r0 + a + 1
                            nc.scalar.sign(
                                xpad[:, rr : rr + N_SIGNROWS, 1 : 1 + W],
                                raw[:, a : a + N_SIGNROWS, :],
                            )
                    if n == 0:
                        border_memsets(xpads[1])
                    # compute: groups of GROUP chunks, ldweights amortized
                    stages = {}
                    for g0 in range(0, n_chunks, GROUP):
                        gs = list(range(g0, g0 + GROUP))
                        pss = {}
                        for g in gs:
                            pss[g] = psum_pool.tile(
                                [C, 4, W], F32, tag="ps", bufs=8, name="ps"
                            )
                        for t in range(6):
                            kw = t % 3
                            wtile = (
                                wdr[:, kw, :, :] if t < 3 else w2z[:, kw, :, :]
                            )
                            nc.tensor.ldweights(wtile, perf_mode=DR)
                            for g in gs:
                                base_row = g * 4 + (0 if t < 3 else 2)
                                base = xpad[:, base_row, kw]
                                rhs = bass.AP(
                                    tensor=base.tensor,
                                    offset=base.offset,
                                    ap=[base.ap[0], [RP, 2], [RP, 4], [1, W]],
                                )
                                mm = nc.tensor.matmul(
                                    pss[g][:, :, :],
                                    wtile,
                                    rhs,
                                    start=(t == 0),
                                    stop=(t == 5),
                                    perf_mode=DR,
                                )
                                mm.ldweights = False
                        for g in gs:
                            s_idx = g // 7
                            jr = (g % 7) * 4
                            if g % 7 == 0:
                                stages[s_idx] = stage_pool.tile(
                                    [C, N_STAGEROWS, W], F16, tag="stage",
                                    bufs=3, name="stage",
                                )
                            nc.vector.tensor_scalar_mul(
                                stages[s_idx][:, jr : jr + 4, :],
                                pss[g][:, :, :],
                                scale[:, :],
                            )
                            if g % 7 == 6:
                                s0 = s_idx * N_STAGEROWS
                                last = n == BL - 1 and g == n_chunks - 1
                                if last:
                                    hs = N_STAGEROWS // 2
                                    nc.gpsimd.dma_start(
                                        yim[:, s0 : s0 + hs, :],
                                        stages[s_idx][:, :hs, :],
                                    )
                                    nc.gpsimd.dma_start(
                                        yim[:, s0 + hs : s0 + N_STAGEROWS, :],
                                        stages[s_idx][:, hs:, :],
                                    )
                                else:
                                    nc.gpsimd.dma_start(
                                        yim[:, s0 : s0 + N_STAGEROWS, :],
                                        stages[s_idx][:, :, :],
                                    )

    nc.compile()
    return nc


_NC_CACHE = {}


def _get_nc(variant=None):
    variant = variant or VARIANT
    if variant not in _NC_CACHE:
        _NC_CACHE[variant] = build_nc_v2(variant)
    return _NC_CACHE[variant]


def kernel(
    x: np.ndarray,
    weight: np.ndarray,
    _trace: bool = False,
    _variant: str | None = None,
    **_kw,
):
    assert x.shape == (B, C, H, W) and weight.shape == (C, C, 3, 3)
    nc = _get_nc(_variant)
    xs = np.ascontiguousarray(x, dtype=np.float32)
    wgt = np.ascontiguousarray(weight, dtype=np.float32)
    in_maps = [
        {"x": xs[i * BL : (i + 1) * BL], "weight": wgt} for i in range(N_CORES)
    ]
    res = run_bass_kernel_spmd(
        nc, in_maps, core_ids=list(range(N_CORES)), trace=_trace
    )
    out = np.concatenate(
        [np.asarray(res.results[i]["y"]) for i in range(N_CORES)], axis=0
    ).astype(np.float32)
    if _trace:
        kernel.last_results = res
    return out
